# revision 14
# baseline (speedup 1.0000x reference)
"""Trainium2 Bass kernel for nn_Attention_46248207844114.

Fused multi-head attention layer (qkv projection + RoPE + causal softmax
attention + output projection), sharded over 8 NeuronCores: data parallel
over batch (2) x tensor parallel over heads (16 -> 4 per core).

Layout strategy: everything on-chip lives "transposed" (feature on the
partition dim, tokens on the free dim), so that:
  - qkv projection emits q^T / k^T directly:  psum[col, n] = Wblk^T @ x^T
  - scores are computed as S^T[k, q] = (k^T)^T-block @ q^T
  - P^T = exp(S^T) feeds the PV matmul as the moving operand with the
    natural-layout v block as the stationary operand
  - output projection emits out^T[e, n], un-transposed on the host
All matmuls run in float32r (full PE rate at >=256 moving columns).
The head-dim of q/k is de-interleaved (even hd first, odd hd second) by
permuting Wq/Wk columns on the host so RoPE's pair rotation becomes a
half-swap of partitions (done with two tiny sbuf->sbuf DMAs).
The softmax denominator is accumulated on the vector engine and reduced
across partitions with a ones-vector matmul; the reciprocal is broadcast
back over partitions with a K=1 matmul.
"""

import sys

if "/opt/trn_rl_repo" not in sys.path:
    sys.path.insert(0, "/opt/trn_rl_repo")

import math

import numpy as np

import concourse.bass as bass
import concourse.mybir as mybir
import concourse.tile as tile
from concourse import bacc, bass_utils

B, N, D, H = 2, 2048, 2048, 16
HD = 128          # head dim
HPC = 4           # heads per core
NCORES = 8
NT = N // 128     # 16 key tiles
DT = D // 128     # 16 contraction tiles
QR = 512          # query range per psum bank
NQR = N // QR     # 4 query ranges
F32 = mybir.dt.float32
F32R = mybir.dt.float32r
NEG = -1e30
SCALE = 1.0 / math.sqrt(HD)

_CACHE = {}


def _build_program():
    nc = bacc.Bacc("TRN2", target_bir_lowering=False, debug=False)

    xT = nc.dram_tensor("xT", [D, N], F32, kind="ExternalInput").ap()
    wqk = nc.dram_tensor("wqk", [D, 2 * HPC * HD], F32, kind="ExternalInput").ap()
    bqk = nc.dram_tensor("bqk", [HD, 2 * HPC], F32, kind="ExternalInput").ap()
    wv = nc.dram_tensor("wv", [D, HPC * HD], F32, kind="ExternalInput").ap()
    bv = nc.dram_tensor("bv", [1, HPC * HD], F32, kind="ExternalInput").ap()
    wout = nc.dram_tensor("wout", [HPC * HD, D], F32, kind="ExternalInput").ap()
    # cos/sin tables, already duplicated to 128 partitions and sign-folded:
    # rows 0:64 cos_i / -sin_i ; rows 64:128 cos_i / +sin_i
    cosq = nc.dram_tensor("cosq", [128, N], F32, kind="ExternalInput").ap()
    sinq = nc.dram_tensor("sinq", [128, N], F32, kind="ExternalInput").ap()
    cosk = nc.dram_tensor("cosk", [128, N], F32, kind="ExternalInput").ap()
    sink = nc.dram_tensor("sink", [128, N], F32, kind="ExternalInput").ap()
    mb = nc.dram_tensor("mb", [128, NT], F32, kind="ExternalInput").ap()
    tm = nc.dram_tensor("tm", [128, 896], F32, kind="ExternalInput").ap()
    onesc = nc.dram_tensor("onesc", [128, 1], F32, kind="ExternalInput").ap()
    onesr = nc.dram_tensor("onesr", [1, 128], F32, kind="ExternalInput").ap()

    outT = nc.dram_tensor("outT", [D, N], F32, kind="ExternalOutput").ap()
    kcache = nc.dram_tensor("kcache", [HPC, HD, N], F32, kind="ExternalOutput").ap()
    vcache = nc.dram_tensor("vcache", [N, HPC * HD], F32, kind="ExternalOutput").ap()

    with tile.TileContext(nc) as tc:
        with (
            tc.tile_pool(name="const", bufs=1) as constp,
            tc.tile_pool(name="dram", bufs=1, space="DRAM") as drp,
        ):
            tm_sb = constp.tile([128, 896], F32R, tag="tm")
            nc.sync.dma_start(out=tm_sb[:], in_=tm.bitcast(F32R))
            mb_sb = constp.tile([128, NT], F32, tag="mb")
            nc.sync.dma_start(out=mb_sb[:], in_=mb)
            bqk_sb = constp.tile([HD, 2 * HPC], F32, tag="bqk")
            nc.sync.dma_start(out=bqk_sb[:], in_=bqk)
            bv_sb = constp.tile([1, HPC * HD], F32R, tag="bv")
            nc.sync.dma_start(out=bv_sb[:], in_=bv.bitcast(F32R))
            ones_col = constp.tile([128, 1], F32R, tag="onc")
            nc.sync.dma_start(out=ones_col[:], in_=onesc.bitcast(F32R))
            ones_row = constp.tile([1, 128], F32R, tag="onr")
            nc.sync.dma_start(out=ones_row[:], in_=onesr.bitcast(F32R))

            # PE warm-up: ~48 cheap matmuls off the first-loaded const tile keep
            # the PE busy while the x/W bulk DMA lands, so HAM reaches K=8/8
            # before the first real matmul chain.
            with tc.tile_pool(name="warm", bufs=2, space="PSUM") as warmp:
                for wi in range(48):
                    wps = warmp.tile([1, 512], F32, tag="wrm", name="wps")
                    nc.tensor.matmul(
                        wps[:], ones_col[:], tm_sb[:, 0:512], start=True, stop=True
                    )

            # DRAM-staged activations (q^T,k^T rope'd; v natural layout).
            # Split per-ct / per-half so attention's loads only depend on the
            # specific projection stores they read (Tile deps are per-tile).
            qk_dram = [
                drp.tile([128, N], F32R, tag=f"qkdram{ct}", name=f"qkdram{ct}")
                for ct in range(2 * HPC)
            ]
            v_dram = [
                drp.tile([128, NT // 2, QR], F32R, tag=f"vdram{hf}", name=f"vdram{hf}")
                for hf in range(2)
            ]

            # ---------------- projection phase ----------------
            with (
                tc.tile_pool(name="xp", bufs=1) as xp,
                tc.tile_pool(name="wp", bufs=3) as wp,
                tc.tile_pool(name="wvp", bufs=4) as wvp,
                tc.tile_pool(name="tbl", bufs=1) as tblp,
                tc.tile_pool(name="pb", bufs=2) as pbp,
                tc.tile_pool(name="psq", bufs=4, space="PSUM") as psq,
                tc.tile_pool(name="psv", bufs=4, space="PSUM") as psv,
            ):
                def emit_v_group(half, grp, xh):
                    vps = []
                    for j in range(4):
                        vp_ = psv.tile([128, 512], F32, tag=f"vps{j}", name=f"vps{j}", bufs=1)
                        vps.append(vp_)
                    dorder = list(range(DT)) if grp == 0 else list(range(DT - 1, -1, -1))
                    wvt_cache = {}
                    for di, d in enumerate(dorder):
                        q4 = d // 4
                        if q4 not in wvt_cache:
                            wv4 = wvp.tile([128, 4, 512], F32R, tag="wv", name="wvt")
                            nc.sync.dma_start(
                                out=wv4[:],
                                in_=wv[q4 * 512 : (q4 + 1) * 512, :]
                                .rearrange("(d p) c -> p d c", p=128)
                                .bitcast(F32R),
                            )
                            wvt_cache[q4] = wv4
                        wvt = wvt_cache[q4][:, d % 4, :]
                        for j in range(4):
                            nt_loc = grp * 4 + j
                            nc.tensor.matmul(
                                vps[j][:],
                                xh[d][:, nt_loc * 128 : nt_loc * 128 + 128],
                                wvt,
                                start=(di == 0),
                                stop=False,
                            )
                    for j in range(4):
                        ntile = half * 8 + grp * 4 + j
                        nc.tensor.matmul(
                            vps[j][:], ones_row[:], bv_sb[:], start=False, stop=True
                        )
                        vsb = pbp.tile([128, 512], F32, tag="vsb", name="vsb", bufs=3)
                        nc.scalar.activation(
                            out=vsb[:],
                            in_=vps[j][:],
                            func=mybir.ActivationFunctionType.Copy,
                            scale=1.0,
                        )
                        nc.gpsimd.dma_start(
                            out=vcache[ntile * 128 : (ntile + 1) * 128, :], in_=vsb[:]
                        )
                        nc.gpsimd.dma_start(
                            out=v_dram[ntile // 8][:, ntile % 8, :], in_=vsb[:].bitcast(F32R)
                        )

                def emit_w(ct):
                    # one 1MB DMA: all 16 [128,128] d-blocks of this ct column
                    wt_ = wp.tile([128, DT, 128], F32R, tag="wct", name="wct")
                    nc.sync.dma_start(
                        out=wt_[:],
                        in_=wqk[:, ct * 128 : (ct + 1) * 128]
                        .rearrange("(d p) c -> p d c", p=128)
                        .bitcast(F32R),
                    )
                    return [wt_[:, d, :] for d in range(DT)]

                for half in range(2):
                    h0 = half * 1024
                    xh = []
                    wcts = {}

                    def ensure_w(c):
                        if c < 2 * HPC and c not in wcts:
                            wcts[c] = emit_w(c)

                    ensure_w(0)
                    for d in range(DT):
                        if d == 5:
                            ensure_w(4)
                        if d == 10:
                            ensure_w(1)
                        xt_ = xp.tile(
                            [128, 1024], F32R, tag=f"x{d}", name=f"xh{d}",
                            bufs=2 if d < 8 else 1,
                        )
                        nc.sync.dma_start(
                            out=xt_[:], in_=xT[d * 128 : (d + 1) * 128, h0 : h0 + 1024].bitcast(F32R)
                        )
                        xh.append(xt_)
                    # rope tables for this half
                    cq = tblp.tile([128, 1024], F32, tag="cq", name="cq")
                    sq = tblp.tile([128, 1024], F32, tag="sq", name="sq")
                    ck = tblp.tile([128, 1024], F32, tag="ck", name="ck")
                    sk = tblp.tile([128, 1024], F32, tag="sk", name="sk")
                    nc.sync.dma_start(out=cq[:], in_=cosq[:, h0 : h0 + 1024])
                    nc.sync.dma_start(out=sq[:], in_=sinq[:, h0 : h0 + 1024])
                    nc.sync.dma_start(out=ck[:], in_=cosk[:, h0 : h0 + 1024])
                    nc.sync.dma_start(out=sk[:], in_=sink[:, h0 : h0 + 1024])

                    # ---- q^T / k^T heads, order q0,k0,q1,k1,... so head h's
                    # q and k finish early and attention can overlap the tail;
                    # v groups interleaved mid-stream ----
                    ct_order = [0, 4, 1, 5, 2, 6, 3, 7]
                    for cti, ct in enumerate(ct_order):
                        ensure_w(ct)
                        if cti + 1 < len(ct_order):
                            ensure_w(ct_order[cti + 1])
                        if cti + 2 < len(ct_order):
                            ensure_w(ct_order[cti + 2])
                        wblk = wcts.pop(ct)
                        for nr in range(2):
                            g0 = h0 + nr * 512
                            ps = psq.tile([128, 512], F32, tag="psqk", name="psqk")
                            for d in range(DT):
                                nc.tensor.matmul(
                                    ps[:],
                                    wblk[d][:],
                                    xh[d][:, nr * 512 : nr * 512 + 512],
                                    start=(d == 0),
                                    stop=(d == DT - 1),
                                )
                            is_q = ct < HPC
                            ctab = cq if is_q else ck
                            stab = sq if is_q else sk
                            b0 = pbp.tile([128, 512], F32, tag="b0", name="b0", bufs=3)
                            nc.scalar.activation(
                                out=b0[:],
                                in_=ps[:],
                                func=mybir.ActivationFunctionType.Identity,
                                bias=bqk_sb[:, ct : ct + 1],
                                scale=1.0,
                            )
                            if not is_q:
                                nc.gpsimd.dma_start(
                                    out=kcache[ct - HPC, :, g0 : g0 + 512], in_=b0[:]
                                )
                            rot = pbp.tile([128, 512], F32, tag="rot", name="rot", bufs=3)
                            nc.gpsimd.dma_start(out=rot[0:64, :], in_=b0[64:128, :])
                            nc.gpsimd.dma_start(out=rot[64:128, :], in_=b0[0:64, :])
                            ro = pbp.tile([128, 512], F32R, tag="ro", name="ro", bufs=3)
                            nc.vector.tensor_mul(ro[:], b0[:], ctab[:, nr * 512 : nr * 512 + 512])
                            u = pbp.tile([128, 512], F32, tag="u", name="u", bufs=3)
                            nc.vector.tensor_mul(u[:], rot[:], stab[:, nr * 512 : nr * 512 + 512])
                            nc.vector.tensor_add(ro[:], ro[:].bitcast(F32), u[:])
                            nc.gpsimd.dma_start(out=qk_dram[ct][:, g0 : g0 + 512], in_=ro[:])
                        if cti == 3 or cti == 7:
                            emit_v_group(half, cti // 4, xh)

            # ---------------- attention phase ----------------
            # j-outer / kt-inner, software-pipelined by LAG on the PE:
            # scores+exp run LAG blocks ahead of PV+denominator so the PE
            # never waits on the activation engine.  The softmax denominator
            # accumulates on the PE (ones-vector matmul into its own psum
            # bank), keeping the vector engine load light.
            with (
                tc.tile_pool(name="attn", bufs=1) as attnp,
                tc.tile_pool(name="wop", bufs=8) as wop,
                tc.tile_pool(name="obp", bufs=6) as obp,
                tc.tile_pool(name="qs", bufs=3) as qsp,
                tc.tile_pool(name="ks", bufs=8) as ksp,
                tc.tile_pool(name="vs", bufs=8) as vsp,
                tc.tile_pool(name="pp", bufs=6) as ppool,
                tc.tile_pool(name="rbp", bufs=2) as rbp,
                tc.tile_pool(name="rcp", bufs=2) as rcp,
                tc.tile_pool(name="pss", bufs=3, space="PSUM") as pss,
                tc.tile_pool(name="psa", bufs=2, space="PSUM") as psa,
                tc.tile_pool(name="psd", bufs=2, space="PSUM") as psd,
            ):
                attn_sc = attnp.tile([128, HPC, N], F32R, tag="attnsc")
                LAG = 2
                for h in range(HPC):
                    for j in range(NQR):
                        nkt = 4 * j + 4
                        qt = qsp.tile([128, 512], F32R, tag="qs", name="qt")
                        nc.sync.dma_start(
                            out=qt[:], in_=qk_dram[h][:, j * 512 : (j + 1) * 512]
                        )
                        a_ps = psa.tile([128, 512], F32, tag="aps", name="aps")
                        d_ps = psd.tile([1, 512], F32, tag="dps", name="dps")
                        pts = {}
                        for kt in range(nkt + LAG):
                            if kt < nkt:
                                ktile = ksp.tile([128, 128], F32R, tag="kt", name="ktile")
                                nc.sync.dma_start(
                                    out=ktile[:],
                                    in_=qk_dram[HPC + h][:, kt * 128 : (kt + 1) * 128],
                                )
                                s_ps = pss.tile([128, 512], F32, tag="sps", name="sps")
                                nc.tensor.matmul(
                                    s_ps[:], ktile[:], qt[:], start=True, stop=True
                                )
                                pt = ppool.tile([128, 512], F32R, tag="P", name="ptile")
                                nc.scalar.activation(
                                    out=pt[:],
                                    in_=s_ps[:],
                                    func=mybir.ActivationFunctionType.Exp,
                                    bias=mb_sb[:, kt : kt + 1],
                                    scale=SCALE,
                                )
                                delta = kt * 128 - j * 512
                                if delta >= 0:
                                    nc.vector.tensor_mul(
                                        pt[:], pt[:], tm_sb[:, 384 - delta : 896 - delta]
                                    )
                                pts[kt] = pt
                            kd = kt - LAG
                            if kd >= 0 and kd < nkt:
                                pt = pts.pop(kd)
                                vt = vsp.tile([128, 128], F32R, tag="vt", name="vt")
                                nc.sync.dma_start(
                                    out=vt[:],
                                    in_=v_dram[kd // 8][:, kd % 8, h * 128 : (h + 1) * 128],
                                )
                                nc.tensor.matmul(
                                    a_ps[:],
                                    vt[:],
                                    pt[:],
                                    start=(kd == 0),
                                    stop=(kd == nkt - 1),
                                )
                                nc.tensor.matmul(
                                    d_ps[:],
                                    ones_col[:],
                                    pt[:],
                                    start=(kd == 0),
                                    stop=(kd == nkt - 1),
                                )
                        # finalize j: reciprocal of denominator, broadcast over
                        # partitions via K=1 matmul, scale the PV output.
                        recip = rcp.tile([1, 512], F32R, tag="recip", name="recip")
                        with nc.allow_low_precision(reason="f32r view of f32 reciprocal"):
                            nc.vector.reciprocal(recip[:], d_ps[:])
                        bc_ps = pss.tile([128, 512], F32, tag="sps", name="bcps")
                        nc.tensor.matmul(bc_ps[:], ones_row[:], recip[:], start=True, stop=True)
                        rb = rbp.tile([128, 512], F32, tag="rb", name="rb")
                        nc.vector.tensor_copy(rb[:], bc_ps[:])
                        nc.vector.tensor_mul(
                            attn_sc[:, h, j * 512 : (j + 1) * 512], a_ps[:], rb[:]
                        )

                # ------------- output projection (psum accumulators reuse the
                # aps/dps pool tags so no pool-close barrier) -------------
                for e in range(DT):
                    for jp in range(2):
                        ops = []
                        for jj in range(2):
                            j = 2 * jp + jj
                            pool = psa if jp == 0 else psd
                            op_ = pool.tile([128, 512], F32, tag=("aps" if jp == 0 else "dps"), name=f"ops{j}")
                            ops.append(op_)
                        for d in range(HPC):
                            wo = wop.tile([128, 128], F32R, tag="wo", name="wo")
                            nc.sync.dma_start(
                                out=wo[:],
                                in_=wout[d * 128 : (d + 1) * 128, e * 128 : (e + 1) * 128].bitcast(F32R),
                            )
                            for jj in range(2):
                                j = 2 * jp + jj
                                nc.tensor.matmul(
                                    ops[jj][:],
                                    wo[:],
                                    attn_sc[:, d, j * 512 : (j + 1) * 512],
                                    start=(d == 0),
                                    stop=(d == HPC - 1),
                                )
                        for jj in range(2):
                            j = 2 * jp + jj
                            ob = obp.tile([128, 512], F32, tag="ob", name="ob")
                            nc.scalar.activation(
                                out=ob[:],
                                in_=ops[jj][:],
                                func=mybir.ActivationFunctionType.Copy,
                                scale=1.0,
                            )
                            eng = nc.sync if (e + j) % 2 == 0 else nc.gpsimd
                            eng.dma_start(
                                out=outT[e * 128 : (e + 1) * 128, j * 512 : (j + 1) * 512],
                                in_=ob[:],
                            )

    nc.compile()
    return nc


_PERM = np.concatenate([np.arange(0, HD, 2), np.arange(1, HD, 2)])


def _host_prep(inputs):
    """Build the 8 per-core input maps from the full problem inputs."""
    x = np.asarray(inputs["x"], dtype=np.float32)
    fq = np.asarray(inputs["freq_cis_q"], dtype=np.float32)
    fk = np.asarray(inputs["freq_cis_k"], dtype=np.float32)
    eam = np.asarray(inputs["expanded_attn_masks"])
    Wqkv = np.asarray(inputs["Wqkv"], dtype=np.float32)
    bqkv = np.asarray(inputs["bqkv"], dtype=np.float32)
    Wout = np.asarray(inputs["Wout"], dtype=np.float32)

    def tables(freqs):
        # freqs [N, 64] -> cos/sin duplicated to 128 rows; sin sign-folded.
        c = np.cos(freqs.T).astype(np.float32)  # [64, N]
        s = np.sin(freqs.T).astype(np.float32)
        cdup = np.concatenate([c, c], axis=0)           # [128, N]
        sdup = np.concatenate([-s, s], axis=0)          # [128, N]
        return np.ascontiguousarray(cdup), np.ascontiguousarray(sdup)

    cq, sq = tables(fq)
    ck, sk = tables(fk)

    # causal multiplicative table: tm[kp, c] = 1.0 if kp <= c-384 else 0.0
    kp = np.arange(128)[:, None]
    cc = np.arange(896)[None, :]
    tm = (kp <= (cc - 384)).astype(np.float32)
    tm = np.ascontiguousarray(tm)

    in_maps = []
    for c in range(NCORES):
        b = c // 4
        g = c % 4
        heads = list(range(4 * g, 4 * g + 4))
        xTb = np.ascontiguousarray(x[b].T)  # [D, N]

        wq_cols = [Wqkv[:, 0 * D + h * HD : 0 * D + (h + 1) * HD][:, _PERM] for h in heads]
        wk_cols = [Wqkv[:, 1 * D + h * HD : 1 * D + (h + 1) * HD][:, _PERM] for h in heads]
        wqk = np.ascontiguousarray(np.concatenate(wq_cols + wk_cols, axis=1))  # [D, 1024]

        bq = [bqkv[0 * D + h * HD : 0 * D + (h + 1) * HD][_PERM] for h in heads]
        bk = [bqkv[1 * D + h * HD : 1 * D + (h + 1) * HD][_PERM] for h in heads]
        bqk = np.ascontiguousarray(np.stack(bq + bk, axis=1))  # [128, 8]

        v0 = 2 * D + 4 * g * HD
        wv = np.ascontiguousarray(Wqkv[:, v0 : v0 + 4 * HD])   # [D, 512]
        bv = np.ascontiguousarray(bqkv[v0 : v0 + 4 * HD][None, :])  # [1, 512]

        wo = np.ascontiguousarray(Wout[4 * g * HD : 4 * (g + 1) * HD, :])  # [512, D]

        m = (eam[b, 0, 0, :] != 0)
        mbias = np.where(m, 0.0, NEG).astype(np.float32).reshape(NT, 128).T
        mbias = np.ascontiguousarray(mbias)  # [128, NT]

        in_maps.append(
            {
                "xT": xTb,
                "wqk": wqk,
                "bqk": bqk,
                "wv": wv,
                "bv": bv,
                "wout": wo,
                "cosq": cq,
                "sinq": sq,
                "cosk": ck,
                "sink": sk,
                "mb": mbias,
                "tm": tm,
                "onesc": np.ones((128, 1), dtype=np.float32),
                "onesr": np.ones((1, 128), dtype=np.float32),
            }
        )
    return in_maps


def _unshard(results, inputs):
    bout = np.asarray(inputs["bout"], dtype=np.float32)
    out = np.zeros((B, N, D), dtype=np.float32)
    kv = np.zeros((2, B, H, N, HD), dtype=np.float32)
    for c in range(NCORES):
        b = c // 4
        g = c % 4
        r = results[c]
        out[b] += r["outT"].T
        for hl in range(HPC):
            h = 4 * g + hl
            kperm = r["kcache"][hl]          # [HD(perm), N]
            knat = np.empty((N, HD), dtype=np.float32)
            knat[:, _PERM] = kperm.T         # undo column permutation
            kv[0, b, h] = knat
            kv[1, b, h] = r["vcache"][:, hl * HD : (hl + 1) * HD]
    out += bout
    return out, kv


def _get_program():
    if "nc" not in _CACHE:
        _CACHE["nc"] = _build_program()
    return _CACHE["nc"]


def kernel(**inputs):
    nc = _get_program()
    in_maps = _host_prep(inputs)
    res = bass_utils.run_bass_kernel_spmd(nc, in_maps, core_ids=list(range(NCORES)))
    return _unshard(res.results, inputs)


def run_traced(**inputs):
    """Like kernel() but returns (outputs, BassKernelResults) with trace."""
    nc = _get_program()
    in_maps = _host_prep(inputs)
    res = bass_utils.run_bass_kernel_spmd(
        nc, in_maps, core_ids=list(range(NCORES)), trace=True
    )
    return _unshard(res.results, inputs), res


# revision 16
# speedup vs baseline: 1.1605x; 1.1605x over previous
"""Trainium2 Bass kernel for nn_Attention_46248207844114.

Fused multi-head attention layer (qkv projection + RoPE + causal softmax
attention + output projection), sharded over 8 NeuronCores: data parallel
over batch (2) x tensor parallel over heads (16 -> 4 per core).

Layout strategy: everything on-chip lives "transposed" (feature on the
partition dim, tokens on the free dim), so that:
  - qkv projection emits q^T / k^T directly:  psum[col, n] = Wblk^T @ x^T
  - scores are computed as S^T[k, q] = (k^T)^T-block @ q^T
  - P^T = exp(S^T) feeds the PV matmul as the moving operand with the
    natural-layout v block as the stationary operand
  - output projection emits out^T[e, n], un-transposed on the host
All matmuls run in float32r (full PE rate at >=256 moving columns).
The head-dim of q/k is de-interleaved (even hd first, odd hd second) by
permuting Wq/Wk columns on the host so RoPE's pair rotation becomes a
half-swap of partitions (done with two tiny sbuf->sbuf DMAs).
The softmax denominator is accumulated on the vector engine and reduced
across partitions with a ones-vector matmul; the reciprocal is broadcast
back over partitions with a K=1 matmul.
"""

import sys

if "/opt/trn_rl_repo" not in sys.path:
    sys.path.insert(0, "/opt/trn_rl_repo")

import math

import numpy as np

import concourse.bass as bass
import concourse.mybir as mybir
import concourse.tile as tile
from concourse import bacc, bass_utils

B, N, D, H = 2, 2048, 2048, 16
HD = 128          # head dim
HPC = 4           # heads per core
NCORES = 8
NT = N // 128     # 16 key tiles
DT = D // 128     # 16 contraction tiles
QR = 512          # query range per psum bank
NQR = N // QR     # 4 query ranges
F32 = mybir.dt.float32
F32R = mybir.dt.float32r
NEG = -1e30
SCALE = 1.0 / math.sqrt(HD)

_CACHE = {}


def _build_program():
    nc = bacc.Bacc("TRN2", target_bir_lowering=False, debug=False)

    xT = nc.dram_tensor("xT", [D, N], F32, kind="ExternalInput").ap()
    wqk = nc.dram_tensor("wqk", [D, 2 * HPC * HD], F32, kind="ExternalInput").ap()
    bqk = nc.dram_tensor("bqk", [HD, 2 * HPC], F32, kind="ExternalInput").ap()
    wv = nc.dram_tensor("wv", [D, HPC * HD], F32, kind="ExternalInput").ap()
    bv = nc.dram_tensor("bv", [1, HPC * HD], F32, kind="ExternalInput").ap()
    wout = nc.dram_tensor("wout", [HPC * HD, D], F32, kind="ExternalInput").ap()
    # cos/sin tables, already duplicated to 128 partitions and sign-folded:
    # rows 0:64 cos_i / -sin_i ; rows 64:128 cos_i / +sin_i
    cosq = nc.dram_tensor("cosq", [128, N], F32, kind="ExternalInput").ap()
    sinq = nc.dram_tensor("sinq", [128, N], F32, kind="ExternalInput").ap()
    cosk = nc.dram_tensor("cosk", [128, N], F32, kind="ExternalInput").ap()
    sink = nc.dram_tensor("sink", [128, N], F32, kind="ExternalInput").ap()
    mb = nc.dram_tensor("mb", [128, NT], F32, kind="ExternalInput").ap()
    tm = nc.dram_tensor("tm", [128, 896], F32, kind="ExternalInput").ap()
    onesc = nc.dram_tensor("onesc", [128, 1], F32, kind="ExternalInput").ap()
    onesr = nc.dram_tensor("onesr", [1, 128], F32, kind="ExternalInput").ap()

    outT = nc.dram_tensor("outT", [D, N], F32, kind="ExternalOutput").ap()
    kcache = nc.dram_tensor("kcache", [HPC, HD, N], F32, kind="ExternalOutput").ap()
    vcache = nc.dram_tensor("vcache", [N, HPC * HD], F32, kind="ExternalOutput").ap()

    with tile.TileContext(nc) as tc:
        with (
            tc.tile_pool(name="const", bufs=1) as constp,
            tc.tile_pool(name="dram", bufs=1, space="DRAM") as drp,
        ):
            tm_sb = constp.tile([128, 896], F32R, tag="tm")
            nc.sync.dma_start(out=tm_sb[:], in_=tm.bitcast(F32R))
            mb_sb = constp.tile([128, NT], F32, tag="mb")
            nc.sync.dma_start(out=mb_sb[:], in_=mb)
            bqk_sb = constp.tile([HD, 2 * HPC], F32, tag="bqk")
            nc.sync.dma_start(out=bqk_sb[:], in_=bqk)
            bv_sb = constp.tile([1, HPC * HD], F32R, tag="bv")
            nc.sync.dma_start(out=bv_sb[:], in_=bv.bitcast(F32R))
            ones_col = constp.tile([128, 1], F32R, tag="onc")
            nc.sync.dma_start(out=ones_col[:], in_=onesc.bitcast(F32R))
            ones_row = constp.tile([1, 128], F32R, tag="onr")
            nc.sync.dma_start(out=ones_row[:], in_=onesr.bitcast(F32R))

            # PE warm-up: ~48 cheap matmuls off the first-loaded const tile keep
            # the PE busy while the x/W bulk DMA lands, so HAM reaches K=8/8
            # before the first real matmul chain.
            with tc.tile_pool(name="warm", bufs=2, space="PSUM") as warmp:
                for wi in range(48):
                    wps = warmp.tile([1, 512], F32, tag="wrm", name="wps")
                    nc.tensor.matmul(
                        wps[:], ones_col[:], tm_sb[:, 0:512], start=True, stop=True
                    )

            # DRAM-staged activations (q^T,k^T rope'd; v natural layout).
            # Split per-ct / per-half so attention's loads only depend on the
            # specific projection stores they read (Tile deps are per-tile).
            qk_dram = [
                drp.tile([128, N], F32R, tag=f"qkdram{ct}", name=f"qkdram{ct}")
                for ct in range(2 * HPC)
            ]
            v_dram = [
                drp.tile([128, NT // 2, QR], F32R, tag=f"vdram{hf}", name=f"vdram{hf}")
                for hf in range(2)
            ]

            # ---------------- projection phase ----------------
            with (
                tc.tile_pool(name="xp", bufs=1) as xp,
                tc.tile_pool(name="wp", bufs=3) as wp,
                tc.tile_pool(name="wvp", bufs=4) as wvp,
                tc.tile_pool(name="tbl", bufs=1) as tblp,
                tc.tile_pool(name="pb", bufs=2) as pbp,
                tc.tile_pool(name="psq", bufs=4, space="PSUM") as psq,
                tc.tile_pool(name="psv", bufs=4, space="PSUM") as psv,
            ):
                def emit_v_group(half, grp, xh):
                    vps = []
                    for j in range(4):
                        vp_ = psv.tile([128, 512], F32, tag=f"vps{j}", name=f"vps{j}", bufs=1)
                        vps.append(vp_)
                    dorder = list(range(DT)) if grp == 0 else list(range(DT - 1, -1, -1))
                    wvt_cache = {}
                    for di, d in enumerate(dorder):
                        q4 = d // 4
                        if q4 not in wvt_cache:
                            wv4 = wvp.tile([128, 4, 512], F32R, tag="wv", name="wvt")
                            nc.sync.dma_start(
                                out=wv4[:],
                                in_=wv[q4 * 512 : (q4 + 1) * 512, :]
                                .rearrange("(d p) c -> p d c", p=128)
                                .bitcast(F32R),
                            )
                            wvt_cache[q4] = wv4
                        wvt = wvt_cache[q4][:, d % 4, :]
                        for j in range(4):
                            nt_loc = grp * 4 + j
                            nc.tensor.matmul(
                                vps[j][:],
                                xh[d][:, nt_loc * 128 : nt_loc * 128 + 128],
                                wvt,
                                start=(di == 0),
                                stop=False,
                            )
                    for j in range(4):
                        ntile = half * 8 + grp * 4 + j
                        nc.tensor.matmul(
                            vps[j][:], ones_row[:], bv_sb[:], start=False, stop=True
                        )
                        vsb = pbp.tile([128, 512], F32, tag="vsb", name="vsb", bufs=3)
                        nc.scalar.activation(
                            out=vsb[:],
                            in_=vps[j][:],
                            func=mybir.ActivationFunctionType.Copy,
                            scale=1.0,
                        )
                        nc.gpsimd.dma_start(
                            out=vcache[ntile * 128 : (ntile + 1) * 128, :], in_=vsb[:]
                        )
                        nc.gpsimd.dma_start(
                            out=v_dram[ntile // 8][:, ntile % 8, :], in_=vsb[:].bitcast(F32R)
                        )

                def emit_w(ct):
                    # one 1MB DMA: all 16 [128,128] d-blocks of this ct column
                    wt_ = wp.tile([128, DT, 128], F32R, tag="wct", name="wct")
                    nc.sync.dma_start(
                        out=wt_[:],
                        in_=wqk[:, ct * 128 : (ct + 1) * 128]
                        .rearrange("(d p) c -> p d c", p=128)
                        .bitcast(F32R),
                    )
                    return [wt_[:, d, :] for d in range(DT)]

                for half in range(2):
                    h0 = half * 1024
                    xh = []
                    wcts = {}

                    def ensure_w(c):
                        if c < 2 * HPC and c not in wcts:
                            wcts[c] = emit_w(c)

                    ensure_w(0)
                    for d in range(DT):
                        if d == 5:
                            ensure_w(4)
                        if d == 10:
                            ensure_w(1)
                        xt_ = xp.tile(
                            [128, 1024], F32R, tag=f"x{d}", name=f"xh{d}",
                            bufs=2 if d < 8 else 1,
                        )
                        nc.sync.dma_start(
                            out=xt_[:], in_=xT[d * 128 : (d + 1) * 128, h0 : h0 + 1024].bitcast(F32R)
                        )
                        xh.append(xt_)
                    # rope tables for this half
                    cq = tblp.tile([128, 1024], F32, tag="cq", name="cq")
                    sq = tblp.tile([128, 1024], F32, tag="sq", name="sq")
                    ck = tblp.tile([128, 1024], F32, tag="ck", name="ck")
                    sk = tblp.tile([128, 1024], F32, tag="sk", name="sk")
                    nc.sync.dma_start(out=cq[:], in_=cosq[:, h0 : h0 + 1024])
                    nc.sync.dma_start(out=sq[:], in_=sinq[:, h0 : h0 + 1024])
                    nc.sync.dma_start(out=ck[:], in_=cosk[:, h0 : h0 + 1024])
                    nc.sync.dma_start(out=sk[:], in_=sink[:, h0 : h0 + 1024])

                    # ---- q^T / k^T heads, order q0,k0,q1,k1,... so head h's
                    # q and k finish early and attention can overlap the tail;
                    # v groups interleaved mid-stream ----
                    ct_order = [0, 4, 1, 5, 2, 6, 3, 7]
                    for cti, ct in enumerate(ct_order):
                        ensure_w(ct)
                        if cti + 1 < len(ct_order):
                            ensure_w(ct_order[cti + 1])
                        if cti + 2 < len(ct_order):
                            ensure_w(ct_order[cti + 2])
                        wblk = wcts.pop(ct)
                        for nr in range(2):
                            g0 = h0 + nr * 512
                            ps = psq.tile([128, 512], F32, tag="psqk", name="psqk")
                            for d in range(DT):
                                nc.tensor.matmul(
                                    ps[:],
                                    wblk[d][:],
                                    xh[d][:, nr * 512 : nr * 512 + 512],
                                    start=(d == 0),
                                    stop=(d == DT - 1),
                                )
                            is_q = ct < HPC
                            ctab = cq if is_q else ck
                            stab = sq if is_q else sk
                            b0 = pbp.tile([128, 512], F32, tag="b0", name="b0", bufs=3)
                            nc.scalar.activation(
                                out=b0[:],
                                in_=ps[:],
                                func=mybir.ActivationFunctionType.Identity,
                                bias=bqk_sb[:, ct : ct + 1],
                                scale=1.0,
                            )
                            if not is_q:
                                nc.gpsimd.dma_start(
                                    out=kcache[ct - HPC, :, g0 : g0 + 512], in_=b0[:]
                                )
                            rot = pbp.tile([128, 512], F32, tag="rot", name="rot", bufs=3)
                            nc.gpsimd.dma_start(out=rot[0:64, :], in_=b0[64:128, :])
                            nc.gpsimd.dma_start(out=rot[64:128, :], in_=b0[0:64, :])
                            ro = pbp.tile([128, 512], F32R, tag="ro", name="ro", bufs=3)
                            nc.vector.tensor_mul(ro[:], b0[:], ctab[:, nr * 512 : nr * 512 + 512])
                            u = pbp.tile([128, 512], F32, tag="u", name="u", bufs=3)
                            nc.vector.tensor_mul(u[:], rot[:], stab[:, nr * 512 : nr * 512 + 512])
                            nc.vector.tensor_add(ro[:], ro[:].bitcast(F32), u[:])
                            nc.gpsimd.dma_start(out=qk_dram[ct][:, g0 : g0 + 512], in_=ro[:])
                        if cti == 3 or cti == 7:
                            emit_v_group(half, cti // 4, xh)

            # ---------------- attention phase ----------------
            # j-outer / kt-inner, software-pipelined by LAG on the PE:
            # scores+exp run LAG blocks ahead of PV+denominator so the PE
            # never waits on the activation engine.  The softmax denominator
            # accumulates on the PE (ones-vector matmul into its own psum
            # bank), keeping the vector engine load light.
            with (
                tc.tile_pool(name="attn", bufs=1) as attnp,
                tc.tile_pool(name="wop", bufs=8) as wop,
                tc.tile_pool(name="obp", bufs=6) as obp,
                tc.tile_pool(name="qs", bufs=3) as qsp,
                tc.tile_pool(name="ks", bufs=2) as ksp,
                tc.tile_pool(name="vs", bufs=2) as vsp,
                tc.tile_pool(name="pp", bufs=6) as ppool,
                tc.tile_pool(name="rbp", bufs=2) as rbp,
                tc.tile_pool(name="rcp", bufs=2) as rcp,
                tc.tile_pool(name="pss", bufs=3, space="PSUM") as pss,
                tc.tile_pool(name="psa", bufs=2, space="PSUM") as psa,
                tc.tile_pool(name="psd", bufs=2, space="PSUM") as psd,
            ):
                attn_sc = attnp.tile([128, HPC, N], F32R, tag="attnsc")
                LAG = 2
                pending_fin = [None]

                def emit_fin():
                    if pending_fin[0] is not None:
                        pending_fin[0]()
                        pending_fin[0] = None

                for h in range(HPC):
                    for j in range(NQR):
                        nkt = 4 * j + 4
                        qt = qsp.tile([128, 512], F32R, tag="qs", name="qt")
                        nc.sync.dma_start(
                            out=qt[:], in_=qk_dram[h][:, j * 512 : (j + 1) * 512]
                        )
                        # batched k / v loads for this (h, j)
                        kb = ksp.tile([128, nkt * 128], F32R, tag="kb", name="kb")
                        nc.sync.dma_start(out=kb[:], in_=qk_dram[HPC + h][:, 0 : nkt * 128])
                        vbs = []
                        for hf in range(2):
                            ntiles = min(max(nkt - 8 * hf, 0), 8)
                            if ntiles == 0:
                                break
                            vb = vsp.tile([128, 8, 128], F32R, tag=f"vb{hf}", name=f"vb{hf}")
                            nc.sync.dma_start(
                                out=vb[:, 0:ntiles, :],
                                in_=v_dram[hf][:, 0:ntiles, h * 128 : (h + 1) * 128],
                            )
                            vbs.append(vb)
                        a_ps = psa.tile([128, 512], F32, tag="aps", name="aps")
                        d_ps = psd.tile([1, 512], F32, tag="dps", name="dps")
                        pts = {}
                        for kt in range(nkt + LAG):
                            if kt < nkt:
                                s_ps = pss.tile([128, 512], F32, tag="sps", name="sps")
                                nc.tensor.matmul(
                                    s_ps[:],
                                    kb[:, kt * 128 : (kt + 1) * 128],
                                    qt[:],
                                    start=True,
                                    stop=True,
                                )
                                pt = ppool.tile([128, 512], F32R, tag="P", name="ptile")
                                nc.scalar.activation(
                                    out=pt[:],
                                    in_=s_ps[:],
                                    func=mybir.ActivationFunctionType.Exp,
                                    bias=mb_sb[:, kt : kt + 1],
                                    scale=SCALE,
                                )
                                delta = kt * 128 - j * 512
                                if delta >= 0:
                                    nc.vector.tensor_mul(
                                        pt[:], pt[:], tm_sb[:, 384 - delta : 896 - delta]
                                    )
                                pts[kt] = pt
                            if kt == 3:
                                emit_fin()  # previous (h, j) finalization
                            kd = kt - LAG
                            if kd >= 0 and kd < nkt:
                                pt = pts.pop(kd)
                                nc.tensor.matmul(
                                    a_ps[:],
                                    vbs[kd // 8][:, kd % 8, :],
                                    pt[:],
                                    start=(kd == 0),
                                    stop=(kd == nkt - 1),
                                )
                                nc.tensor.matmul(
                                    d_ps[:],
                                    ones_col[:],
                                    pt[:],
                                    start=(kd == 0),
                                    stop=(kd == nkt - 1),
                                )

                        def make_fin(h=h, j=j, a_ps=a_ps, d_ps=d_ps):
                            def fin():
                                recip = rcp.tile([1, 512], F32R, tag="recip", name="recip")
                                with nc.allow_low_precision(reason="f32r recip view"):
                                    nc.vector.reciprocal(recip[:], d_ps[:])
                                bc_ps = pss.tile([128, 512], F32, tag="sps", name="bcps")
                                nc.tensor.matmul(
                                    bc_ps[:], ones_row[:], recip[:], start=True, stop=True
                                )
                                rb = rbp.tile([128, 512], F32, tag="rb", name="rb")
                                nc.vector.tensor_copy(rb[:], bc_ps[:])
                                nc.vector.tensor_mul(
                                    attn_sc[:, h, j * 512 : (j + 1) * 512], a_ps[:], rb[:]
                                )
                            return fin

                        pending_fin[0] = make_fin()
                emit_fin()

                # ------------- output projection (psum accumulators reuse the
                # aps/dps pool tags so no pool-close barrier) -------------
                for e in range(DT):
                    for jp in range(2):
                        ops = []
                        for jj in range(2):
                            j = 2 * jp + jj
                            pool = psa if jp == 0 else psd
                            op_ = pool.tile([128, 512], F32, tag=("aps" if jp == 0 else "dps"), name=f"ops{j}")
                            ops.append(op_)
                        for d in range(HPC):
                            wo = wop.tile([128, 128], F32R, tag="wo", name="wo")
                            nc.sync.dma_start(
                                out=wo[:],
                                in_=wout[d * 128 : (d + 1) * 128, e * 128 : (e + 1) * 128].bitcast(F32R),
                            )
                            for jj in range(2):
                                j = 2 * jp + jj
                                nc.tensor.matmul(
                                    ops[jj][:],
                                    wo[:],
                                    attn_sc[:, d, j * 512 : (j + 1) * 512],
                                    start=(d == 0),
                                    stop=(d == HPC - 1),
                                )
                        for jj in range(2):
                            j = 2 * jp + jj
                            ob = obp.tile([128, 512], F32, tag="ob", name="ob")
                            nc.scalar.activation(
                                out=ob[:],
                                in_=ops[jj][:],
                                func=mybir.ActivationFunctionType.Copy,
                                scale=1.0,
                            )
                            eng = nc.sync if (e + j) % 2 == 0 else nc.gpsimd
                            eng.dma_start(
                                out=outT[e * 128 : (e + 1) * 128, j * 512 : (j + 1) * 512],
                                in_=ob[:],
                            )

    nc.compile()
    return nc


_PERM = np.concatenate([np.arange(0, HD, 2), np.arange(1, HD, 2)])


def _host_prep(inputs):
    """Build the 8 per-core input maps from the full problem inputs."""
    x = np.asarray(inputs["x"], dtype=np.float32)
    fq = np.asarray(inputs["freq_cis_q"], dtype=np.float32)
    fk = np.asarray(inputs["freq_cis_k"], dtype=np.float32)
    eam = np.asarray(inputs["expanded_attn_masks"])
    Wqkv = np.asarray(inputs["Wqkv"], dtype=np.float32)
    bqkv = np.asarray(inputs["bqkv"], dtype=np.float32)
    Wout = np.asarray(inputs["Wout"], dtype=np.float32)

    def tables(freqs):
        # freqs [N, 64] -> cos/sin duplicated to 128 rows; sin sign-folded.
        c = np.cos(freqs.T).astype(np.float32)  # [64, N]
        s = np.sin(freqs.T).astype(np.float32)
        cdup = np.concatenate([c, c], axis=0)           # [128, N]
        sdup = np.concatenate([-s, s], axis=0)          # [128, N]
        return np.ascontiguousarray(cdup), np.ascontiguousarray(sdup)

    cq, sq = tables(fq)
    ck, sk = tables(fk)

    # causal multiplicative table: tm[kp, c] = 1.0 if kp <= c-384 else 0.0
    kp = np.arange(128)[:, None]
    cc = np.arange(896)[None, :]
    tm = (kp <= (cc - 384)).astype(np.float32)
    tm = np.ascontiguousarray(tm)

    in_maps = []
    for c in range(NCORES):
        b = c // 4
        g = c % 4
        heads = list(range(4 * g, 4 * g + 4))
        xTb = np.ascontiguousarray(x[b].T)  # [D, N]

        wq_cols = [Wqkv[:, 0 * D + h * HD : 0 * D + (h + 1) * HD][:, _PERM] for h in heads]
        wk_cols = [Wqkv[:, 1 * D + h * HD : 1 * D + (h + 1) * HD][:, _PERM] for h in heads]
        wqk = np.ascontiguousarray(np.concatenate(wq_cols + wk_cols, axis=1))  # [D, 1024]

        bq = [bqkv[0 * D + h * HD : 0 * D + (h + 1) * HD][_PERM] for h in heads]
        bk = [bqkv[1 * D + h * HD : 1 * D + (h + 1) * HD][_PERM] for h in heads]
        bqk = np.ascontiguousarray(np.stack(bq + bk, axis=1))  # [128, 8]

        v0 = 2 * D + 4 * g * HD
        wv = np.ascontiguousarray(Wqkv[:, v0 : v0 + 4 * HD])   # [D, 512]
        bv = np.ascontiguousarray(bqkv[v0 : v0 + 4 * HD][None, :])  # [1, 512]

        wo = np.ascontiguousarray(Wout[4 * g * HD : 4 * (g + 1) * HD, :])  # [512, D]

        m = (eam[b, 0, 0, :] != 0)
        mbias = np.where(m, 0.0, NEG).astype(np.float32).reshape(NT, 128).T
        mbias = np.ascontiguousarray(mbias)  # [128, NT]

        in_maps.append(
            {
                "xT": xTb,
                "wqk": wqk,
                "bqk": bqk,
                "wv": wv,
                "bv": bv,
                "wout": wo,
                "cosq": cq,
                "sinq": sq,
                "cosk": ck,
                "sink": sk,
                "mb": mbias,
                "tm": tm,
                "onesc": np.ones((128, 1), dtype=np.float32),
                "onesr": np.ones((1, 128), dtype=np.float32),
            }
        )
    return in_maps


def _unshard(results, inputs):
    bout = np.asarray(inputs["bout"], dtype=np.float32)
    out = np.zeros((B, N, D), dtype=np.float32)
    kv = np.zeros((2, B, H, N, HD), dtype=np.float32)
    for c in range(NCORES):
        b = c // 4
        g = c % 4
        r = results[c]
        out[b] += r["outT"].T
        for hl in range(HPC):
            h = 4 * g + hl
            kperm = r["kcache"][hl]          # [HD(perm), N]
            knat = np.empty((N, HD), dtype=np.float32)
            knat[:, _PERM] = kperm.T         # undo column permutation
            kv[0, b, h] = knat
            kv[1, b, h] = r["vcache"][:, hl * HD : (hl + 1) * HD]
    out += bout
    return out, kv


def _get_program():
    if "nc" not in _CACHE:
        _CACHE["nc"] = _build_program()
    return _CACHE["nc"]


def kernel(**inputs):
    nc = _get_program()
    in_maps = _host_prep(inputs)
    res = bass_utils.run_bass_kernel_spmd(nc, in_maps, core_ids=list(range(NCORES)))
    return _unshard(res.results, inputs)


def run_traced(**inputs):
    """Like kernel() but returns (outputs, BassKernelResults) with trace."""
    nc = _get_program()
    in_maps = _host_prep(inputs)
    res = bass_utils.run_bass_kernel_spmd(
        nc, in_maps, core_ids=list(range(NCORES)), trace=True
    )
    return _unshard(res.results, inputs), res


# revision 17
# speedup vs baseline: 1.2424x; 1.0705x over previous
"""Trainium2 Bass kernel for nn_Attention_46248207844114.

Fused multi-head attention layer (qkv projection + RoPE + causal softmax
attention + output projection), sharded over 8 NeuronCores: data parallel
over batch (2) x tensor parallel over heads (16 -> 4 per core).

Layout strategy: everything on-chip lives "transposed" (feature on the
partition dim, tokens on the free dim), so that:
  - qkv projection emits q^T / k^T directly:  psum[col, n] = Wblk^T @ x^T
  - scores are computed as S^T[k, q] = (k^T)^T-block @ q^T
  - P^T = exp(S^T) feeds the PV matmul as the moving operand with the
    natural-layout v block as the stationary operand
  - output projection emits out^T[e, n], un-transposed on the host
All matmuls run in float32r (full PE rate at >=256 moving columns).
The head-dim of q/k is de-interleaved (even hd first, odd hd second) by
permuting Wq/Wk columns on the host so RoPE's pair rotation becomes a
half-swap of partitions (done with two tiny sbuf->sbuf DMAs).
The softmax denominator is accumulated on the vector engine and reduced
across partitions with a ones-vector matmul; the reciprocal is broadcast
back over partitions with a K=1 matmul.
"""

import sys

if "/opt/trn_rl_repo" not in sys.path:
    sys.path.insert(0, "/opt/trn_rl_repo")

import math

import numpy as np

import concourse.bass as bass
import concourse.mybir as mybir
import concourse.tile as tile
from concourse import bacc, bass_utils

B, N, D, H = 2, 2048, 2048, 16
HD = 128          # head dim
HPC = 4           # heads per core
NCORES = 8
NT = N // 128     # 16 key tiles
DT = D // 128     # 16 contraction tiles
QR = 512          # query range per psum bank
NQR = N // QR     # 4 query ranges
F32 = mybir.dt.float32
F32R = mybir.dt.float32r
NEG = -1e30
SCALE = 1.0 / math.sqrt(HD)

_CACHE = {}


def _build_program():
    nc = bacc.Bacc("TRN2", target_bir_lowering=False, debug=False)

    xT = nc.dram_tensor("xT", [D, N], F32, kind="ExternalInput").ap()
    wqk = nc.dram_tensor("wqk", [D, 2 * HPC * HD], F32, kind="ExternalInput").ap()
    bqk = nc.dram_tensor("bqk", [HD, 2 * HPC], F32, kind="ExternalInput").ap()
    wv = nc.dram_tensor("wv", [D, HPC * HD], F32, kind="ExternalInput").ap()
    bv = nc.dram_tensor("bv", [1, HPC * HD], F32, kind="ExternalInput").ap()
    wout = nc.dram_tensor("wout", [HPC * HD, D], F32, kind="ExternalInput").ap()
    # cos/sin tables, already duplicated to 128 partitions and sign-folded:
    # rows 0:64 cos_i / -sin_i ; rows 64:128 cos_i / +sin_i
    cosq = nc.dram_tensor("cosq", [128, N], F32, kind="ExternalInput").ap()
    sinq = nc.dram_tensor("sinq", [128, N], F32, kind="ExternalInput").ap()
    cosk = nc.dram_tensor("cosk", [128, N], F32, kind="ExternalInput").ap()
    sink = nc.dram_tensor("sink", [128, N], F32, kind="ExternalInput").ap()
    mb = nc.dram_tensor("mb", [128, NT], F32, kind="ExternalInput").ap()
    tm = nc.dram_tensor("tm", [128, 896], F32, kind="ExternalInput").ap()
    onesc = nc.dram_tensor("onesc", [128, 1], F32, kind="ExternalInput").ap()
    onesr = nc.dram_tensor("onesr", [1, 128], F32, kind="ExternalInput").ap()

    outT = nc.dram_tensor("outT", [D, N], F32, kind="ExternalOutput").ap()
    kcache = nc.dram_tensor("kcache", [HPC, HD, N], F32, kind="ExternalOutput").ap()
    vcache = nc.dram_tensor("vcache", [N, HPC * HD], F32, kind="ExternalOutput").ap()

    with tile.TileContext(nc) as tc:
        with (
            tc.tile_pool(name="const", bufs=1) as constp,
            tc.tile_pool(name="dram", bufs=1, space="DRAM") as drp,
        ):
            tm_sb = constp.tile([128, 896], F32R, tag="tm")
            nc.sync.dma_start(out=tm_sb[:], in_=tm.bitcast(F32R))
            mb_sb = constp.tile([128, NT], F32, tag="mb")
            nc.sync.dma_start(out=mb_sb[:], in_=mb)
            bqk_sb = constp.tile([HD, 2 * HPC], F32, tag="bqk")
            nc.sync.dma_start(out=bqk_sb[:], in_=bqk)
            bv_sb = constp.tile([1, HPC * HD], F32R, tag="bv")
            nc.sync.dma_start(out=bv_sb[:], in_=bv.bitcast(F32R))
            ones_col = constp.tile([128, 1], F32R, tag="onc")
            nc.sync.dma_start(out=ones_col[:], in_=onesc.bitcast(F32R))
            ones_row = constp.tile([1, 128], F32R, tag="onr")
            nc.sync.dma_start(out=ones_row[:], in_=onesr.bitcast(F32R))

            # PE warm-up: keep the PE busy from t~0 while the x/W bulk DMA
            # lands so HAM reaches K=8/8 before the first real matmul chain.
            # memset-sourced fp32 matmuls avoid any DMA dependency.
            with (
                tc.tile_pool(name="warmsb", bufs=1) as warmsb,
                tc.tile_pool(name="warm", bufs=2, space="PSUM") as warmp,
            ):
                wl = warmsb.tile([128, 1], F32, tag="wl")
                nc.vector.memset(wl[:], 1.0)
                wr = warmsb.tile([128, 512], F32, tag="wr")
                nc.vector.memset(wr[:], 1.0)
                for wi in range(24):
                    wps = warmp.tile([1, 512], F32, tag="wrm", name="wps")
                    nc.tensor.matmul(wps[:], wl[:], wr[:], start=True, stop=True)

            # DRAM-staged activations (q^T,k^T rope'd; v natural layout).
            # Split per-ct / per-half so attention's loads only depend on the
            # specific projection stores they read (Tile deps are per-tile).
            qk_dram = [
                drp.tile([128, N], F32R, tag=f"qkdram{ct}", name=f"qkdram{ct}")
                for ct in range(2 * HPC)
            ]
            v_dram = [
                drp.tile([128, NT // 2, QR], F32R, tag=f"vdram{hf}", name=f"vdram{hf}")
                for hf in range(2)
            ]

            # ---------------- projection phase ----------------
            with (
                tc.tile_pool(name="xp", bufs=1) as xp,
                tc.tile_pool(name="wp", bufs=4) as wp,
                tc.tile_pool(name="wvp", bufs=4) as wvp,
                tc.tile_pool(name="tbl", bufs=1) as tblp,
                tc.tile_pool(name="pb", bufs=2) as pbp,
                tc.tile_pool(name="psq", bufs=4, space="PSUM") as psq,
                tc.tile_pool(name="psv", bufs=4, space="PSUM") as psv,
            ):
                def emit_v_group(half, grp, xh):
                    vps = []
                    for j in range(4):
                        vp_ = psv.tile([128, 512], F32, tag=f"vps{j}", name=f"vps{j}", bufs=1)
                        vps.append(vp_)
                    dorder = list(range(DT)) if grp == 0 else list(range(DT - 1, -1, -1))
                    wvt_cache = {}
                    for di, d in enumerate(dorder):
                        q4 = d // 4
                        if q4 not in wvt_cache:
                            wv4 = wvp.tile([128, 4, 512], F32R, tag="wv", name="wvt")
                            nc.sync.dma_start(
                                out=wv4[:],
                                in_=wv[q4 * 512 : (q4 + 1) * 512, :]
                                .rearrange("(d p) c -> p d c", p=128)
                                .bitcast(F32R),
                            )
                            wvt_cache[q4] = wv4
                        wvt = wvt_cache[q4][:, d % 4, :]
                        for j in range(4):
                            nt_loc = grp * 4 + j
                            nc.tensor.matmul(
                                vps[j][:],
                                xh[d][:, nt_loc * 128 : nt_loc * 128 + 128],
                                wvt,
                                start=(di == 0),
                                stop=False,
                            )
                    for j in range(4):
                        ntile = half * 8 + grp * 4 + j
                        nc.tensor.matmul(
                            vps[j][:], ones_row[:], bv_sb[:], start=False, stop=True
                        )
                        vsb = pbp.tile([128, 512], F32, tag="vsb", name="vsb", bufs=3)
                        nc.scalar.activation(
                            out=vsb[:],
                            in_=vps[j][:],
                            func=mybir.ActivationFunctionType.Copy,
                            scale=1.0,
                        )
                        nc.gpsimd.dma_start(
                            out=vcache[ntile * 128 : (ntile + 1) * 128, :], in_=vsb[:]
                        )
                        nc.gpsimd.dma_start(
                            out=v_dram[ntile // 8][:, ntile % 8, :], in_=vsb[:].bitcast(F32R)
                        )

                def emit_w(ct):
                    # one 1MB DMA: all 16 [128,128] d-blocks of this ct column
                    wt_ = wp.tile([128, DT, 128], F32R, tag="wct", name="wct")
                    nc.sync.dma_start(
                        out=wt_[:],
                        in_=wqk[:, ct * 128 : (ct + 1) * 128]
                        .rearrange("(d p) c -> p d c", p=128)
                        .bitcast(F32R),
                    )
                    return [wt_[:, d, :] for d in range(DT)]

                for half in range(2):
                    h0 = half * 1024
                    xh = []
                    wcts = {}

                    def ensure_w(c):
                        if c < 2 * HPC and c not in wcts:
                            wcts[c] = emit_w(c)

                    ensure_w(0)
                    for d in range(DT):
                        if d == 5:
                            ensure_w(4)
                        if d == 10:
                            ensure_w(1)
                        xt_ = xp.tile(
                            [128, 1024], F32R, tag=f"x{d}", name=f"xh{d}",
                            bufs=2 if d < 6 else 1,
                        )
                        nc.sync.dma_start(
                            out=xt_[:], in_=xT[d * 128 : (d + 1) * 128, h0 : h0 + 1024].bitcast(F32R)
                        )
                        xh.append(xt_)
                    # rope tables for this half
                    cq = tblp.tile([128, 1024], F32, tag="cq", name="cq")
                    sq = tblp.tile([128, 1024], F32, tag="sq", name="sq")
                    ck = tblp.tile([128, 1024], F32, tag="ck", name="ck")
                    sk = tblp.tile([128, 1024], F32, tag="sk", name="sk")
                    nc.sync.dma_start(out=cq[:], in_=cosq[:, h0 : h0 + 1024])
                    nc.sync.dma_start(out=sq[:], in_=sinq[:, h0 : h0 + 1024])
                    nc.sync.dma_start(out=ck[:], in_=cosk[:, h0 : h0 + 1024])
                    nc.sync.dma_start(out=sk[:], in_=sink[:, h0 : h0 + 1024])

                    # ---- q^T / k^T heads, order q0,k0,q1,k1,... so head h's
                    # q and k finish early and attention can overlap the tail;
                    # v groups interleaved mid-stream ----
                    ct_order = [0, 4, 1, 5, 2, 6, 3, 7]
                    for cti, ct in enumerate(ct_order):
                        ensure_w(ct)
                        if cti + 1 < len(ct_order):
                            ensure_w(ct_order[cti + 1])
                        if cti + 2 < len(ct_order):
                            ensure_w(ct_order[cti + 2])
                        wblk = wcts.pop(ct)
                        for nr in range(2):
                            g0 = h0 + nr * 512
                            ps = psq.tile([128, 512], F32, tag="psqk", name="psqk")
                            for d in range(DT):
                                nc.tensor.matmul(
                                    ps[:],
                                    wblk[d][:],
                                    xh[d][:, nr * 512 : nr * 512 + 512],
                                    start=(d == 0),
                                    stop=(d == DT - 1),
                                )
                            is_q = ct < HPC
                            ctab = cq if is_q else ck
                            stab = sq if is_q else sk
                            b0 = pbp.tile([128, 512], F32, tag="b0", name="b0", bufs=3)
                            nc.scalar.activation(
                                out=b0[:],
                                in_=ps[:],
                                func=mybir.ActivationFunctionType.Identity,
                                bias=bqk_sb[:, ct : ct + 1],
                                scale=1.0,
                            )
                            if not is_q:
                                nc.gpsimd.dma_start(
                                    out=kcache[ct - HPC, :, g0 : g0 + 512], in_=b0[:]
                                )
                            rot = pbp.tile([128, 512], F32, tag="rot", name="rot", bufs=3)
                            nc.gpsimd.dma_start(out=rot[0:64, :], in_=b0[64:128, :])
                            nc.gpsimd.dma_start(out=rot[64:128, :], in_=b0[0:64, :])
                            ro = pbp.tile([128, 512], F32R, tag="ro", name="ro", bufs=3)
                            nc.vector.tensor_mul(ro[:], b0[:], ctab[:, nr * 512 : nr * 512 + 512])
                            u = pbp.tile([128, 512], F32, tag="u", name="u", bufs=3)
                            nc.vector.tensor_mul(u[:], rot[:], stab[:, nr * 512 : nr * 512 + 512])
                            nc.vector.tensor_add(ro[:], ro[:].bitcast(F32), u[:])
                            nc.gpsimd.dma_start(out=qk_dram[ct][:, g0 : g0 + 512], in_=ro[:])
                        if cti == 3 or cti == 7:
                            emit_v_group(half, cti // 4, xh)

            # ---------------- attention phase ----------------
            # j-outer / kt-inner, software-pipelined by LAG on the PE:
            # scores+exp run LAG blocks ahead of PV+denominator so the PE
            # never waits on the activation engine.  The softmax denominator
            # accumulates on the PE (ones-vector matmul into its own psum
            # bank), keeping the vector engine load light.
            with (
                tc.tile_pool(name="attn", bufs=1) as attnp,
                tc.tile_pool(name="wop", bufs=8) as wop,
                tc.tile_pool(name="obp", bufs=6) as obp,
                tc.tile_pool(name="qs", bufs=3) as qsp,
                tc.tile_pool(name="ks", bufs=2) as ksp,
                tc.tile_pool(name="vs", bufs=2) as vsp,
                tc.tile_pool(name="pp", bufs=6) as ppool,
                tc.tile_pool(name="rbp", bufs=2) as rbp,
                tc.tile_pool(name="rcp", bufs=2) as rcp,
                tc.tile_pool(name="pss", bufs=4, space="PSUM") as pss,
                tc.tile_pool(name="psa", bufs=2, space="PSUM") as psa,
                tc.tile_pool(name="psd", bufs=2, space="PSUM") as psd,
            ):
                attn_sc = attnp.tile([128, HPC, N], F32R, tag="attnsc")
                LAG = 3
                pending_fin = [None]

                def emit_fin():
                    if pending_fin[0] is not None:
                        pending_fin[0]()
                        pending_fin[0] = None

                for h in range(HPC):
                    for j in range(NQR):
                        nkt = 4 * j + 4
                        qt = qsp.tile([128, 512], F32R, tag="qs", name="qt")
                        nc.sync.dma_start(
                            out=qt[:], in_=qk_dram[h][:, j * 512 : (j + 1) * 512]
                        )
                        # batched k / v loads for this (h, j)
                        kb = ksp.tile([128, nkt * 128], F32R, tag="kb", name="kb")
                        nc.sync.dma_start(out=kb[:], in_=qk_dram[HPC + h][:, 0 : nkt * 128])
                        vbs = []
                        for hf in range(2):
                            ntiles = min(max(nkt - 8 * hf, 0), 8)
                            if ntiles == 0:
                                break
                            vb = vsp.tile([128, 8, 128], F32R, tag=f"vb{hf}", name=f"vb{hf}")
                            nc.sync.dma_start(
                                out=vb[:, 0:ntiles, :],
                                in_=v_dram[hf][:, 0:ntiles, h * 128 : (h + 1) * 128],
                            )
                            vbs.append(vb)
                        a_ps = psa.tile([128, 512], F32, tag="aps", name="aps")
                        d_ps = psd.tile([1, 512], F32, tag="dps", name="dps")
                        pts = {}
                        for kt in range(nkt + LAG):
                            if kt < nkt:
                                s_ps = pss.tile([128, 512], F32, tag="sps", name="sps")
                                nc.tensor.matmul(
                                    s_ps[:],
                                    kb[:, kt * 128 : (kt + 1) * 128],
                                    qt[:],
                                    start=True,
                                    stop=True,
                                )
                                pt = ppool.tile([128, 512], F32R, tag="P", name="ptile")
                                nc.scalar.activation(
                                    out=pt[:],
                                    in_=s_ps[:],
                                    func=mybir.ActivationFunctionType.Exp,
                                    bias=mb_sb[:, kt : kt + 1],
                                    scale=SCALE,
                                )
                                delta = kt * 128 - j * 512
                                if delta >= 0:
                                    nc.vector.tensor_mul(
                                        pt[:], pt[:], tm_sb[:, 384 - delta : 896 - delta]
                                    )
                                pts[kt] = pt
                            if kt == 3:
                                emit_fin()  # previous (h, j) finalization
                            kd = kt - LAG
                            if kd >= 0 and kd < nkt:
                                pt = pts.pop(kd)
                                nc.tensor.matmul(
                                    a_ps[:],
                                    vbs[kd // 8][:, kd % 8, :],
                                    pt[:],
                                    start=(kd == 0),
                                    stop=(kd == nkt - 1),
                                )
                                nc.tensor.matmul(
                                    d_ps[:],
                                    ones_col[:],
                                    pt[:],
                                    start=(kd == 0),
                                    stop=(kd == nkt - 1),
                                )

                        def make_fin(h=h, j=j, a_ps=a_ps, d_ps=d_ps):
                            def fin():
                                recip = rcp.tile([1, 512], F32R, tag="recip", name="recip")
                                with nc.allow_low_precision(reason="f32r recip view"):
                                    nc.vector.reciprocal(recip[:], d_ps[:])
                                bc_ps = pss.tile([128, 512], F32, tag="sps", name="bcps")
                                nc.tensor.matmul(
                                    bc_ps[:], ones_row[:], recip[:], start=True, stop=True
                                )
                                rb = rbp.tile([128, 512], F32, tag="rb", name="rb")
                                nc.vector.tensor_copy(rb[:], bc_ps[:])
                                nc.vector.tensor_mul(
                                    attn_sc[:, h, j * 512 : (j + 1) * 512], a_ps[:], rb[:]
                                )
                            return fin

                        pending_fin[0] = make_fin()
                emit_fin()

                # ------------- output projection (psum accumulators cycle
                # through the attention pools' aps/dps/sps tags -> ~7 banks in
                # rotation, no pool-close barrier, no bank stalls) -------------
                fin_tags = [(psa, "aps"), (psd, "dps"), (pss, "sps")]
                for e in range(DT):
                    ops = []
                    for j in range(NQR):
                        pool, tag = fin_tags[(e * NQR + j) % 3]
                        op_ = pool.tile([128, 512], F32, tag=tag, name=f"ops{j}")
                        ops.append(op_)
                    for d in range(HPC):
                        wo = wop.tile([128, 128], F32R, tag="wo", name="wo")
                        nc.sync.dma_start(
                            out=wo[:],
                            in_=wout[d * 128 : (d + 1) * 128, e * 128 : (e + 1) * 128].bitcast(F32R),
                        )
                        for j in range(NQR):
                            nc.tensor.matmul(
                                ops[j][:],
                                wo[:],
                                attn_sc[:, d, j * 512 : (j + 1) * 512],
                                start=(d == 0),
                                stop=(d == HPC - 1),
                            )
                    for j in range(NQR):
                        ob = obp.tile([128, 512], F32, tag="ob", name="ob")
                        nc.scalar.activation(
                            out=ob[:],
                            in_=ops[j][:],
                            func=mybir.ActivationFunctionType.Copy,
                            scale=1.0,
                        )
                        eng = nc.sync if (e + j) % 2 == 0 else nc.gpsimd
                        eng.dma_start(
                            out=outT[e * 128 : (e + 1) * 128, j * 512 : (j + 1) * 512],
                            in_=ob[:],
                        )

    nc.compile()
    return nc


_PERM = np.concatenate([np.arange(0, HD, 2), np.arange(1, HD, 2)])


def _host_prep(inputs):
    """Build the 8 per-core input maps from the full problem inputs."""
    x = np.asarray(inputs["x"], dtype=np.float32)
    fq = np.asarray(inputs["freq_cis_q"], dtype=np.float32)
    fk = np.asarray(inputs["freq_cis_k"], dtype=np.float32)
    eam = np.asarray(inputs["expanded_attn_masks"])
    Wqkv = np.asarray(inputs["Wqkv"], dtype=np.float32)
    bqkv = np.asarray(inputs["bqkv"], dtype=np.float32)
    Wout = np.asarray(inputs["Wout"], dtype=np.float32)

    def tables(freqs):
        # freqs [N, 64] -> cos/sin duplicated to 128 rows; sin sign-folded.
        c = np.cos(freqs.T).astype(np.float32)  # [64, N]
        s = np.sin(freqs.T).astype(np.float32)
        cdup = np.concatenate([c, c], axis=0)           # [128, N]
        sdup = np.concatenate([-s, s], axis=0)          # [128, N]
        return np.ascontiguousarray(cdup), np.ascontiguousarray(sdup)

    cq, sq = tables(fq)
    ck, sk = tables(fk)

    # causal multiplicative table: tm[kp, c] = 1.0 if kp <= c-384 else 0.0
    kp = np.arange(128)[:, None]
    cc = np.arange(896)[None, :]
    tm = (kp <= (cc - 384)).astype(np.float32)
    tm = np.ascontiguousarray(tm)

    in_maps = []
    for c in range(NCORES):
        b = c // 4
        g = c % 4
        heads = list(range(4 * g, 4 * g + 4))
        xTb = np.ascontiguousarray(x[b].T)  # [D, N]

        wq_cols = [Wqkv[:, 0 * D + h * HD : 0 * D + (h + 1) * HD][:, _PERM] for h in heads]
        wk_cols = [Wqkv[:, 1 * D + h * HD : 1 * D + (h + 1) * HD][:, _PERM] for h in heads]
        wqk = np.ascontiguousarray(np.concatenate(wq_cols + wk_cols, axis=1))  # [D, 1024]

        bq = [bqkv[0 * D + h * HD : 0 * D + (h + 1) * HD][_PERM] for h in heads]
        bk = [bqkv[1 * D + h * HD : 1 * D + (h + 1) * HD][_PERM] for h in heads]
        bqk = np.ascontiguousarray(np.stack(bq + bk, axis=1))  # [128, 8]

        v0 = 2 * D + 4 * g * HD
        wv = np.ascontiguousarray(Wqkv[:, v0 : v0 + 4 * HD])   # [D, 512]
        bv = np.ascontiguousarray(bqkv[v0 : v0 + 4 * HD][None, :])  # [1, 512]

        wo = np.ascontiguousarray(Wout[4 * g * HD : 4 * (g + 1) * HD, :])  # [512, D]

        m = (eam[b, 0, 0, :] != 0)
        mbias = np.where(m, 0.0, NEG).astype(np.float32).reshape(NT, 128).T
        mbias = np.ascontiguousarray(mbias)  # [128, NT]

        in_maps.append(
            {
                "xT": xTb,
                "wqk": wqk,
                "bqk": bqk,
                "wv": wv,
                "bv": bv,
                "wout": wo,
                "cosq": cq,
                "sinq": sq,
                "cosk": ck,
                "sink": sk,
                "mb": mbias,
                "tm": tm,
                "onesc": np.ones((128, 1), dtype=np.float32),
                "onesr": np.ones((1, 128), dtype=np.float32),
            }
        )
    return in_maps


def _unshard(results, inputs):
    bout = np.asarray(inputs["bout"], dtype=np.float32)
    out = np.zeros((B, N, D), dtype=np.float32)
    kv = np.zeros((2, B, H, N, HD), dtype=np.float32)
    for c in range(NCORES):
        b = c // 4
        g = c % 4
        r = results[c]
        out[b] += r["outT"].T
        for hl in range(HPC):
            h = 4 * g + hl
            kperm = r["kcache"][hl]          # [HD(perm), N]
            knat = np.empty((N, HD), dtype=np.float32)
            knat[:, _PERM] = kperm.T         # undo column permutation
            kv[0, b, h] = knat
            kv[1, b, h] = r["vcache"][:, hl * HD : (hl + 1) * HD]
    out += bout
    return out, kv


def _get_program():
    if "nc" not in _CACHE:
        _CACHE["nc"] = _build_program()
    return _CACHE["nc"]


def kernel(**inputs):
    nc = _get_program()
    in_maps = _host_prep(inputs)
    res = bass_utils.run_bass_kernel_spmd(nc, in_maps, core_ids=list(range(NCORES)))
    return _unshard(res.results, inputs)


def run_traced(**inputs):
    """Like kernel() but returns (outputs, BassKernelResults) with trace."""
    nc = _get_program()
    in_maps = _host_prep(inputs)
    res = bass_utils.run_bass_kernel_spmd(
        nc, in_maps, core_ids=list(range(NCORES)), trace=True
    )
    return _unshard(res.results, inputs), res
